# revision 8
# baseline (speedup 1.0000x reference)
"""Trainium2 Bass kernel for nn_DirectedEdgeMessage (GNN message passing).

Computation per molecule b (B=256, A=64 atoms, E=128 edges, K=6 neighbors,
H=256 features):
  w[e]   = 1 / ||xyz[p0[e]] - xyz[p1[e]]||^2      (0 where distance == 0)
  msg[e] = sum_k w[nb[e,k]] * R[nb[e,k], :]

Strategy (data-parallel over B across 8 NeuronCores, 32 molecules/core):
  * E == 128 == PE array width, so the neighbor gather+sum is a matmul
    msg = S @ R with a per-molecule scatter matrix
    S[e,e'] = w[e'] * |{k : nb[e,k] == e'}|.
  * One-hot rows U_k[e,e'] = (nb[e,k] == e') are built with
    tensor_scalar(is_equal) against a constant iota row (bf16, exact, 4x
    DVE mode); split between DVE and GPSIMD to balance engine load.
  * The PE transposes and K-reduces them via accumulating matmuls
    U_k.T @ I into one PSUM tile = C^T counts (fp32, exact).
  * The per-partition w scale is fused into the PSUM->SBUF copy
    (Activation engine, [128,1] scale AP), output bf16 -> full-rate
    bf16 main matmul against a bf16 R tile (half the HBM traffic of
    fp32).
  * The xyz pair gather is pre-transposed: a host-replicated prb tensor
    (prb[p,(b,e)] = bond_pairs[b,e,side(p)]) lets one tensor_scalar
    per 8 molecules build a SIGNED one-hot lhsT directly
    ((prb == atom_iota) * sign, both per-partition scalars), so
    diff[e,c] = pohT.T @ [xyz_hi; xyz_lo] with no PE transpose and no
    PSUM round-trip.  xyz is split hi/lo in bf16 host-side so the
    bf16 matmul reproduces fp32 coordinates exactly (accumulated in
    fp32 PSUM) -- near-pair distances stay accurate.
  * msg leaves PSUM as bf16 (Act/GPSIMD copies) and is DMAd to HBM as
    bf16; the host widens to fp32.  Measured absmax/scale stays well
    under the 2e-2 gate.
"""

import numpy as np
import ml_dtypes
from contextlib import ExitStack

import concourse.bass as bass
import concourse.tile as tile
from concourse import bacc, mybir
from concourse.bass_utils import run_bass_kernel_spmd

B, A, E, K, H = 256, 64, 128, 6, 256
NCORES = 8
BLOC = B // NCORES   # 32 molecules per core
GRP = 8              # molecules per R-tile DMA group
NGRP = BLOC // GRP
UNIT = 4             # molecules per PSUM msg tile / output DMA

F32 = mybir.dt.float32
BF16 = mybir.dt.bfloat16
I32 = mybir.dt.int32
EQ = mybir.AluOpType.is_equal
GT = mybir.AluOpType.is_gt
MULT = mybir.AluOpType.mult
ADD = mybir.AluOpType.add

# Engine-assignment knobs (tunable):
#   u_pool(b, k) -> True: build one-hot (b,k) on GPSIMD instead of DVE.
#     Even molecules: k4+k5 on Pool folded into one tile (ts+stt chain);
#     odd molecules: only k5 on Pool.  -> DVE 4.5 ops/mol, Pool 1.5.
#   msg_pool_units: which of the 8 msg-copy units go to GPSIMD (rest Act).
CFG = {
    "u_pool_even": (4, 5),   # ks on Pool for even molecules (folded chain)
    "u_pool_odd": (5,),      # ks on Pool for odd molecules
    "msg_dve_units": (3,),   # msg-copy units on DVE (rest Act); Pool can't
    "stw_eng": "act",        # read PSUM, so only Act/DVE are eligible
}


def _emit_pipeline(nc, tc, d, sb, pools):
    """Emit one full pass over the core's 32 molecules."""
    iota_sb, id_sb, nb_bf, prb_sb, xyzcat, i64c, sgnc = (
        sb["iota"], sb["ident"], sb["nb_bf"], sb["prb"], sb["xyzcat"],
        sb["i64c"], sb["sgnc"])
    r_t = d["r"].ap().transpose([1, 0, 2])    # [E, BLOC, H] view
    o_t = d["out"].ap().transpose([1, 0, 2])

    # ---- Phase A: distance weights for all 32 molecules ----
    pohT = pools["poh"].tile([E, BLOC * E], BF16, tag="pohT")
    for g in range(NGRP):
        nc.vector.tensor_scalar(
            pohT[:, g * GRP * E:(g + 1) * GRP * E],
            prb_sb[:, g * GRP * E:(g + 1) * GRP * E],
            i64c[:], sgnc[:], op0=EQ, op1=MULT)
    ps_d = pools["psd"].tile([E, BLOC * 3], F32, tag="psd")
    for b in range(BLOC):
        nc.tensor.matmul(ps_d[:, b * 3:(b + 1) * 3],
                         pohT[:, b * E:(b + 1) * E],
                         xyzcat[:, b * 3:(b + 1) * 3],
                         start=True, stop=False)
        nc.tensor.matmul(ps_d[:, b * 3:(b + 1) * 3],
                         pohT[:, b * E:(b + 1) * E],
                         xyzcat[:, (BLOC + b) * 3:(BLOC + b + 1) * 3],
                         start=False, stop=True)
    sq = pools["sq"].tile([E, BLOC * 3], F32, tag="sq")
    nc.scalar.square(sq[:], ps_d[:])
    d2a = pools["sq"].tile([E, BLOC], F32, tag="d2a")
    nc.vector.tensor_add(d2a[:], sq[:, 0:BLOC * 3:3], sq[:, 1:BLOC * 3:3])
    d2 = pools["sq"].tile([E, BLOC], F32, tag="d2")
    nc.vector.tensor_add(d2[:], d2a[:], sq[:, 2:BLOC * 3:3])
    d2c = pools["sq"].tile([E, BLOC], F32, tag="d2c")
    nc.vector.tensor_scalar_max(d2c[:], d2[:], 1e-20)
    winv = pools["sq"].tile([E, BLOC], F32, tag="winv")
    nc.vector.reciprocal_approx_fast(winv[:], d2c[:])
    w_sb = pools["w"].tile([E, BLOC], F32, tag="w")
    nc.vector.scalar_tensor_tensor(
        w_sb[:], d2[:], 0.0, winv[:], op0=GT, op1=MULT)

    # ---- Phase B: scatter matrices + message matmuls ----
    for g in range(NGRP):
        gb = g * GRP
        r_sb = pools["r"].tile([E, GRP * H], BF16, tag="r")
        nc.sync.dma_start(r_sb[:], r_t[:, gb:gb + GRP, :])
        for half in range(GRP // UNIT):
            unit_idx = g * (GRP // UNIT) + half
            ps_mm = pools["psmm"].tile([E, UNIT * H], F32, tag="psmm")
            for o in range(UNIT):
                bb = half * UNIT + o        # molecule index within group
                b = gb + bb
                pool_ks = (CFG["u_pool_even"] if b % 2 == 0
                           else CFG["u_pool_odd"])
                u = pools["u"].tile([E, K * E], BF16, tag="u")
                for k in range(K):
                    eng = nc.gpsimd if k in pool_ks else nc.vector
                    eng.tensor_scalar(
                        u[:, k * E:(k + 1) * E], iota_sb[:],
                        nb_bf[:, b * K + k:b * K + k + 1], None, op0=EQ)
                ps_st = pools["psst"].tile([E, E], F32, tag="psst")
                for k in range(K):
                    nc.tensor.matmul(ps_st[:], u[:, k * E:(k + 1) * E],
                                     id_sb[:],
                                     start=(k == 0), stop=(k == K - 1))
                stw = pools["stw"].tile([E, E], BF16, tag="stw")
                if CFG["stw_eng"] == "act":
                    nc.scalar.mul(stw[:], ps_st[:], w_sb[:, b:b + 1])
                elif CFG["stw_eng"] == "dve":
                    nc.vector.tensor_scalar(
                        stw[:], ps_st[:], w_sb[:, b:b + 1], None, op0=MULT)
                else:
                    nc.gpsimd.tensor_scalar(
                        stw[:], ps_st[:], w_sb[:, b:b + 1], None, op0=MULT)
                nc.tensor.matmul(ps_mm[:, o * H:(o + 1) * H],
                                 stw[:], r_sb[:, bb * H:(bb + 1) * H],
                                 start=True, stop=True)
            msg_sb = pools["msg"].tile([E, UNIT * H], BF16, tag="msg")
            if unit_idx in CFG["msg_dve_units"]:
                nc.vector.tensor_copy(msg_sb[:], ps_mm[:])
            else:
                nc.scalar.copy(msg_sb[:], ps_mm[:])
            nc.sync.dma_start(
                o_t[:, gb + half * UNIT:gb + (half + 1) * UNIT, :], msg_sb[:])


def build_program(loop_iters=None, body_unroll=8):
    """Build the per-core Bass program. loop_iters=None emits one straight-line
    pass (production). loop_iters=N wraps body_unroll passes in a For_i(0,N)
    device loop — used only for wall-clock timing via iteration deltas."""
    nc = bacc.Bacc("TRN2", target_bir_lowering=False, debug=False)

    d = {
        "r": nc.dram_tensor("r", [BLOC, E, H], BF16, kind="ExternalInput"),
        "nbt": nc.dram_tensor("nbt", [E, BLOC, K], I32, kind="ExternalInput"),
        "prb": nc.dram_tensor("prb", [E, BLOC * E], BF16, kind="ExternalInput"),
        "xyzh": nc.dram_tensor("xyzh", [A, BLOC * 3], BF16,
                               kind="ExternalInput"),
        "xyzl": nc.dram_tensor("xyzl", [A, BLOC * 3], BF16,
                               kind="ExternalInput"),
        "out": nc.dram_tensor("out", [BLOC, E, H], BF16, kind="ExternalOutput"),
    }
    iota_np = np.broadcast_to(np.arange(E, dtype=np.float32), (E, E))
    c_iota = nc.inline_tensor(
        np.ascontiguousarray(iota_np.astype(ml_dtypes.bfloat16)), "c_iota")
    c_id = nc.inline_tensor(
        np.eye(E, dtype=np.float32).astype(ml_dtypes.bfloat16), "c_ident")
    c_i64 = nc.inline_tensor(
        (np.arange(E, dtype=np.float32) % A).reshape(E, 1), "c_i64")
    c_sgn = nc.inline_tensor(
        np.where(np.arange(E) < A, 1.0, -1.0).astype(np.float32).reshape(E, 1),
        "c_sgn")

    with tile.TileContext(nc) as tc, ExitStack() as ctx:
        cpool = ctx.enter_context(tc.tile_pool(name="const", bufs=1))
        prb_sb = cpool.tile([E, BLOC * E], BF16, tag="prb")
        nc.sync.dma_start(prb_sb[:], d["prb"].ap()[:])
        iota_sb = cpool.tile([E, E], BF16, tag="iota")
        nc.scalar.dma_start(iota_sb[:], c_iota.ap()[:])
        nb_i = cpool.tile([E, BLOC * K], I32, tag="nbi")
        nc.sync.dma_start(nb_i[:], d["nbt"].ap()[:])
        id_sb = cpool.tile([E, E], BF16, tag="ident")
        nc.scalar.dma_start(id_sb[:], c_id.ap()[:])
        i64c = cpool.tile([E, 1], F32, tag="i64c")
        nc.scalar.dma_start(i64c[:], c_i64.ap()[:])
        sgnc = cpool.tile([E, 1], F32, tag="sgnc")
        nc.scalar.dma_start(sgnc[:], c_sgn.ap()[:])
        # xyzcat: cols [0:96] = hi per (b,c); cols [96:192] = lo.
        # Atom coords replicated on partitions 0-63 and 64-127 (the signed
        # one-hot handles the +/-).
        xyzcat = cpool.tile([E, BLOC * 6], BF16, tag="xyzcat")
        nc.sync.dma_start(xyzcat[0:A, 0:BLOC * 3], d["xyzh"].ap()[:])
        nc.scalar.dma_start(xyzcat[A:2 * A, 0:BLOC * 3], d["xyzh"].ap()[:])
        nc.sync.dma_start(xyzcat[0:A, BLOC * 3:BLOC * 6], d["xyzl"].ap()[:])
        nc.scalar.dma_start(xyzcat[A:2 * A, BLOC * 3:BLOC * 6],
                            d["xyzl"].ap()[:])

        nb_bf = cpool.tile([E, BLOC * K], F32, tag="nbbf")
        nc.vector.tensor_copy(nb_bf[:], nb_i[:])

        sb = {"iota": iota_sb, "ident": id_sb, "nb_bf": nb_bf,
              "prb": prb_sb, "xyzcat": xyzcat, "i64c": i64c, "sgnc": sgnc}
        pools = {
            "r": ctx.enter_context(tc.tile_pool(name="r", bufs=3)),
            "msg": ctx.enter_context(tc.tile_pool(name="msg", bufs=3)),
            "poh": ctx.enter_context(tc.tile_pool(name="poh", bufs=2)),
            "u": ctx.enter_context(tc.tile_pool(name="u", bufs=8)),
            "stw": ctx.enter_context(tc.tile_pool(name="stw", bufs=6)),
            "sq": ctx.enter_context(tc.tile_pool(name="sq", bufs=2)),
            "w": ctx.enter_context(tc.tile_pool(name="w", bufs=2)),
            "psd": ctx.enter_context(tc.tile_pool(name="psd", bufs=1,
                                                  space="PSUM")),
            "psst": ctx.enter_context(tc.tile_pool(name="psst", bufs=3,
                                                   space="PSUM")),
            "psmm": ctx.enter_context(tc.tile_pool(name="psmm", bufs=2,
                                                   space="PSUM")),
        }
        if loop_iters is None:
            _emit_pipeline(nc, tc, d, sb, pools)
        else:
            with tc.For_i(0, loop_iters, 1,
                          hint_engines=(mybir.EngineType.DVE,
                                        mybir.EngineType.Activation,
                                        mybir.EngineType.PE)):
                for _ in range(body_unroll):
                    _emit_pipeline(nc, tc, d, sb, pools)

    nc.compile()
    return nc


def shard_inputs(bond_representations, bond_pairs, bond_neighbors, xyz):
    in_maps = []
    for c in range(NCORES):
        sl = slice(c * BLOC, (c + 1) * BLOC)
        r = np.ascontiguousarray(bond_representations[0, sl],
                                 dtype=np.float32).astype(ml_dtypes.bfloat16)
        pr = np.asarray(bond_pairs[sl], dtype=np.float32)  # [BLOC, E, 2]
        prb = np.empty((E, BLOC, E), dtype=ml_dtypes.bfloat16)
        prb[0:A] = pr[None, :, :, 0]
        prb[A:E] = pr[None, :, :, 1]
        xyzt = np.ascontiguousarray(
            np.transpose(xyz[sl], (1, 0, 2)), dtype=np.float32)  # [A, BLOC, 3]
        xh = xyzt.astype(ml_dtypes.bfloat16)
        xl = (xyzt - xh.astype(np.float32)).astype(ml_dtypes.bfloat16)
        in_maps.append({
            "r": r,
            "nbt": np.ascontiguousarray(
                np.transpose(bond_neighbors[sl], (1, 0, 2)), dtype=np.int32),
            "prb": np.ascontiguousarray(prb.reshape(E, BLOC * E)),
            "xyzh": np.ascontiguousarray(xh.reshape(A, BLOC * 3)),
            "xyzl": np.ascontiguousarray(xl.reshape(A, BLOC * 3)),
        })
    return in_maps


_PROG_CACHE = {}


def _get_program(key=(None, 8)):
    if key not in _PROG_CACHE:
        _PROG_CACHE[key] = build_program(loop_iters=key[0], body_unroll=key[1])
    return _PROG_CACHE[key]


def kernel(**inputs):
    args = {k: np.asarray(v) for k, v in inputs.items()}
    in_maps = shard_inputs(args["bond_representations"], args["bond_pairs"],
                           args["bond_neighbors"], args["xyz"])
    nc = _get_program()
    res = run_bass_kernel_spmd(nc, in_maps, list(range(NCORES)))
    out = np.concatenate(
        [np.asarray(res.results[c]["out"]).astype(np.float32)
         for c in range(NCORES)], axis=0)
    return out[None]


# revision 26
# speedup vs baseline: 3.6103x; 3.6103x over previous
"""Trainium2 Bass kernel for nn_DirectedEdgeMessage (GNN message passing).

Computation per molecule b (B=256, A=64 atoms, E=128 edges, K=6 neighbors,
H=256 features):
  w[e]   = 1 / ||xyz[p0[e]] - xyz[p1[e]]||^2      (0 where distance == 0)
  msg[e] = sum_k w[nb[e,k]] * R[nb[e,k], :]

Strategy (data-parallel over B across 8 NeuronCores, 32 molecules/core):
  * msg = C^T-partitioned matmuls against w-scaled R: for each molecule,
    msg[e,h] = sum_e' count[e,e'] * (w[e'] R[e',h]).
  * The count matrix is built DIRECTLY TRANSPOSED on the DVE using a
    host-replicated neighbor tensor nbb[p,(b,k,e)] = nb[b,e,k] (same value
    on every partition).  One wide tensor_scalar(is_equal) per k against a
    per-partition iota column gives U_k^T[e',(b,e)] for ALL 32 molecules in
    one 4096-col op (4x DVE mode) -- this replaced 192 narrow per-molecule
    ops AND 192 PE transpose matmuls AND 32 PSUM->SBUF scale-copies from
    the earlier design (GPSIMD measured ~6x slower than modeled on HW, so
    everything runs on DVE/Act/PE only).
  * The K-fold splits between DVE adds (2x mode) and PE accumulation:
    FOLD_P pre-summed count tiles -> FOLD_P accumulating main matmuls.
  * w folds into R (rw = w * R, one 4x DVE op per molecule), not into the
    count matrix, so count tiles stay unscaled/exact.
  * The xyz pair gather is also pre-transposed: prb[p,(b,e)] =
    bond_pairs[b,e,side(p)] lets one tensor_scalar per 8 molecules build a
    SIGNED one-hot lhsT ((prb == atom_iota) * sign), so diff = pohT.T @
    [xyz_hi | xyz_lo] with no PE transpose.  xyz is split hi/lo in bf16
    host-side so fp32 coordinates are reproduced exactly in the PSUM
    accumulate (near-pair distances stay accurate).
  * R input and msg output travel as bf16 (half the HBM traffic); the host
    widens the output to fp32.  Measured absmax/scale ~4e-3, well under
    the 2e-2 gate.
"""

import numpy as np
import ml_dtypes
from contextlib import ExitStack

import concourse.bass as bass
import concourse.tile as tile
from concourse import bacc, mybir
from concourse.bass_utils import run_bass_kernel_spmd

B, A, E, K, H = 256, 64, 128, 6, 256
NCORES = 8
BLOC = B // NCORES   # 32 molecules per core
GRP = 8              # molecules per R-tile DMA group
NGRP = BLOC // GRP
UNIT = 4             # molecules per PSUM msg tile / output DMA

F32 = mybir.dt.float32
BF16 = mybir.dt.bfloat16
EQ = mybir.AluOpType.is_equal
GT = mybir.AluOpType.is_gt
MULT = mybir.AluOpType.mult
ADD = mybir.AluOpType.add

CFG = {
    "fold_p": 2,          # pre-summed count tiles (DVE adds = 6-P, PE mains = P)
    "rw_act_mols": 16,    # molecules whose rw scale runs on Act instead of DVE
    "msg_dve_units": (),  # msg-copy units on DVE (rest Act)
}


def _k_groups():
    p = CFG["fold_p"]
    ks = list(range(K))
    return [ks[i::p] for i in range(p)]


def _emit_pipeline(nc, tc, d, sb, pools):
    """Emit one full pass over the core's 32 molecules."""
    prb_sb, nbb_sb, xyzcat, i64c, sgnc, ecol = (
        sb["prb"], sb["nbb"], sb["xyzcat"], sb["i64c"], sb["sgnc"],
        sb["ecol"])
    r_t = d["r"].ap().transpose([1, 0, 2])    # [E, BLOC, H] view
    o_t = d["out"].ap().transpose([1, 0, 2])

    # ---- Phase A: distance weights for all 32 molecules ----
    pohT = pools["poh"].tile([E, BLOC * E], BF16, tag="pohT")
    for g in range(NGRP):
        nc.vector.tensor_scalar(
            pohT[:, g * GRP * E:(g + 1) * GRP * E],
            prb_sb[:, g * GRP * E:(g + 1) * GRP * E],
            i64c[:], sgnc[:], op0=EQ, op1=MULT)
    ps_d = pools["psd"].tile([E, BLOC, 2, 3], F32, tag="psd")
    for b in range(BLOC):
        nc.tensor.matmul(ps_d[:, b, :, :],
                         pohT[:, b * E:(b + 1) * E],
                         xyzcat[:, b, :],
                         start=True, stop=True)
    hisb = pools["sq"].tile([E, BLOC, 3], F32, tag="hisb")
    nc.scalar.copy(hisb[:], ps_d[:, :, 0, :])
    dsum = pools["sq"].tile([E, BLOC, 3], F32, tag="dsum")
    nc.vector.tensor_add(dsum[:], hisb[:], ps_d[:, :, 1, :])
    sq = pools["sq"].tile([E, BLOC * 3], F32, tag="sq")
    nc.scalar.square(sq[:], dsum[:])
    d2a = pools["sq"].tile([E, BLOC], F32, tag="d2a")
    nc.vector.tensor_add(d2a[:], sq[:, 0:BLOC * 3:3], sq[:, 1:BLOC * 3:3])
    d2 = pools["sq"].tile([E, BLOC], F32, tag="d2")
    nc.vector.tensor_add(d2[:], d2a[:], sq[:, 2:BLOC * 3:3])
    d2c = pools["sq"].tile([E, BLOC], F32, tag="d2c")
    nc.vector.tensor_scalar_max(d2c[:], d2[:], 1e-20)
    winv = pools["sq"].tile([E, BLOC], F32, tag="winv")
    nc.vector.reciprocal_approx_fast(winv[:], d2c[:])
    w_sb = pools["w"].tile([E, BLOC], F32, tag="w")
    nc.vector.scalar_tensor_tensor(
        w_sb[:], d2[:], 0.0, winv[:], op0=GT, op1=MULT)

    # ---- Count tiles: C^T summed over k, built transposed and wide ----
    cts = []
    for gi, ks in enumerate(_k_groups()):
        ct = pools["ct"].tile([E, BLOC, E], BF16, tag="ct")
        if len(ks) == 1:
            nc.vector.tensor_scalar(
                ct[:], nbb_sb[:, :, ks[0], :], ecol[:], None, op0=EQ)
        else:
            oh0 = pools["oh"].tile([E, BLOC, E], BF16, tag="oh")
            nc.vector.tensor_scalar(
                oh0[:], nbb_sb[:, :, ks[0], :], ecol[:], None, op0=EQ)
            for j, k in enumerate(ks[1:]):
                oh1 = pools["oh"].tile([E, BLOC, E], BF16, tag="oh")
                nc.vector.tensor_scalar(
                    oh1[:], nbb_sb[:, :, k, :], ecol[:], None, op0=EQ)
                dst = ct if j == len(ks) - 2 else oh0
                nc.vector.tensor_tensor(dst[:], oh0[:], oh1[:], op=ADD)
        cts.append(ct)

    # ---- Phase B: w-scaled R + message matmuls ----
    for g in range(NGRP):
        gb = g * GRP
        r_sb = pools["r"].tile([E, GRP * H], BF16, tag="r")
        nc.sync.dma_start(r_sb[:], r_t[:, gb:gb + GRP, :])
        rw = pools["rw"].tile([E, GRP * H], BF16, tag="rw")
        for bb in range(GRP):
            b = gb + bb
            if b < CFG["rw_act_mols"]:
                nc.scalar.mul(rw[:, bb * H:(bb + 1) * H],
                              r_sb[:, bb * H:(bb + 1) * H], w_sb[:, b:b + 1])
            else:
                nc.vector.tensor_scalar(
                    rw[:, bb * H:(bb + 1) * H], r_sb[:, bb * H:(bb + 1) * H],
                    w_sb[:, b:b + 1], None, op0=MULT)
        for half in range(GRP // UNIT):
            unit_idx = g * (GRP // UNIT) + half
            ps_mm = pools["psmm"].tile([E, UNIT * H], F32, tag="psmm")
            for o in range(UNIT):
                bb = half * UNIT + o
                b = gb + bb
                for pi, ct in enumerate(cts):
                    nc.tensor.matmul(ps_mm[:, o * H:(o + 1) * H],
                                     ct[:, b, :],
                                     rw[:, bb * H:(bb + 1) * H],
                                     start=(pi == 0), stop=(pi == len(cts) - 1))
            msg_sb = pools["msg"].tile([E, UNIT * H], BF16, tag="msg")
            if unit_idx in CFG["msg_dve_units"]:
                nc.vector.tensor_copy(msg_sb[:], ps_mm[:])
            else:
                nc.scalar.copy(msg_sb[:], ps_mm[:])
            nc.sync.dma_start(
                o_t[:, gb + half * UNIT:gb + (half + 1) * UNIT, :], msg_sb[:])


def build_program(loop_iters=None, body_unroll=8):
    """Build the per-core Bass program. loop_iters=None emits one straight-line
    pass (production). loop_iters=N wraps body_unroll passes in a For_i(0,N)
    device loop — used only for wall-clock timing via iteration deltas."""
    nc = bacc.Bacc("TRN2", target_bir_lowering=False, debug=False)

    d = {
        "r": nc.dram_tensor("r", [BLOC, E, H], BF16, kind="ExternalInput"),
        "nbb": nc.dram_tensor("nbb", [E, BLOC * K * E], BF16,
                              kind="ExternalInput"),
        "prb": nc.dram_tensor("prb", [E, BLOC * E], BF16, kind="ExternalInput"),
        "xyzhl": nc.dram_tensor("xyzhl", [A, BLOC * 6], BF16,
                                kind="ExternalInput"),
        "out": nc.dram_tensor("out", [BLOC, E, H], BF16, kind="ExternalOutput"),
    }
    c_i64 = nc.inline_tensor(
        (np.arange(E, dtype=np.float32) % A).reshape(E, 1), "c_i64")
    c_sgn = nc.inline_tensor(
        np.where(np.arange(E) < A, 1.0, -1.0).astype(np.float32).reshape(E, 1),
        "c_sgn")
    c_ecol = nc.inline_tensor(
        np.arange(E, dtype=np.float32).reshape(E, 1), "c_ecol")

    with tile.TileContext(nc) as tc, ExitStack() as ctx:
        cpool = ctx.enter_context(tc.tile_pool(name="const", bufs=1))
        prb_sb = cpool.tile([E, BLOC * E], BF16, tag="prb")
        nc.sync.dma_start(prb_sb[:], d["prb"].ap()[:])
        nbb_sb = cpool.tile([E, BLOC, K, E], BF16, tag="nbb")
        nc.sync.dma_start(nbb_sb[:], d["nbb"].ap()[:])
        i64c = cpool.tile([E, 1], F32, tag="i64c")
        nc.scalar.dma_start(i64c[:], c_i64.ap()[:])
        sgnc = cpool.tile([E, 1], F32, tag="sgnc")
        nc.scalar.dma_start(sgnc[:], c_sgn.ap()[:])
        ecol = cpool.tile([E, 1], F32, tag="ecol")
        nc.scalar.dma_start(ecol[:], c_ecol.ap()[:])
        # xyzcat[p, b, 0:3] = bf16-hi xyz, [p, b, 3:6] = bf16-lo residual.
        # Atom coords replicated on partitions 0-63 and 64-127 (the signed
        # one-hot handles the +/-).
        xyzcat = cpool.tile([E, BLOC, 6], BF16, tag="xyzcat")
        nc.sync.dma_start(xyzcat[0:A, :, :], d["xyzhl"].ap()[:])
        nc.scalar.dma_start(xyzcat[A:2 * A, :, :], d["xyzhl"].ap()[:])

        sb = {"prb": prb_sb, "nbb": nbb_sb, "xyzcat": xyzcat,
              "i64c": i64c, "sgnc": sgnc, "ecol": ecol}
        pools = {
            "r": ctx.enter_context(tc.tile_pool(name="r", bufs=3)),
            "rw": ctx.enter_context(tc.tile_pool(name="rw", bufs=3)),
            "msg": ctx.enter_context(tc.tile_pool(name="msg", bufs=3)),
            "poh": ctx.enter_context(tc.tile_pool(name="poh", bufs=2)),
            "ct": ctx.enter_context(
                tc.tile_pool(name="ct", bufs=2 * CFG["fold_p"])),
            "oh": ctx.enter_context(tc.tile_pool(name="oh", bufs=3)),
            "sq": ctx.enter_context(tc.tile_pool(name="sq", bufs=2)),
            "w": ctx.enter_context(tc.tile_pool(name="w", bufs=2)),
            "psd": ctx.enter_context(tc.tile_pool(name="psd", bufs=1,
                                                  space="PSUM")),
            "psmm": ctx.enter_context(tc.tile_pool(name="psmm", bufs=3,
                                                   space="PSUM")),
        }
        if loop_iters is None:
            _emit_pipeline(nc, tc, d, sb, pools)
        else:
            with tc.For_i(0, loop_iters, 1,
                          hint_engines=(mybir.EngineType.DVE,
                                        mybir.EngineType.Activation,
                                        mybir.EngineType.PE)):
                for _ in range(body_unroll):
                    _emit_pipeline(nc, tc, d, sb, pools)

    nc.compile()
    return nc


def shard_inputs(bond_representations, bond_pairs, bond_neighbors, xyz):
    in_maps = []
    for c in range(NCORES):
        sl = slice(c * BLOC, (c + 1) * BLOC)
        r = np.ascontiguousarray(bond_representations[0, sl],
                                 dtype=np.float32).astype(ml_dtypes.bfloat16)
        pr = np.asarray(bond_pairs[sl], dtype=np.float32)  # [BLOC, E, 2]
        prb = np.empty((E, BLOC, E), dtype=ml_dtypes.bfloat16)
        prb[0:A] = pr[None, :, :, 0]
        prb[A:E] = pr[None, :, :, 1]
        # nbb[p, b, k, e] = nb[b, e, k], identical on every partition p.
        nbt = np.transpose(np.asarray(bond_neighbors[sl], dtype=np.float32),
                           (0, 2, 1)).astype(ml_dtypes.bfloat16)  # [BLOC,K,E]
        nbb = np.ascontiguousarray(
            np.broadcast_to(nbt[None], (E, BLOC, K, E)))
        xyzt = np.ascontiguousarray(
            np.transpose(xyz[sl], (1, 0, 2)), dtype=np.float32)  # [A, BLOC, 3]
        xh = xyzt.astype(ml_dtypes.bfloat16)
        xl = (xyzt - xh.astype(np.float32)).astype(ml_dtypes.bfloat16)
        xhl = np.concatenate([xh, xl], axis=2)  # [A, BLOC, 6]
        in_maps.append({
            "r": r,
            "nbb": np.ascontiguousarray(nbb.reshape(E, BLOC * K * E)),
            "prb": np.ascontiguousarray(prb.reshape(E, BLOC * E)),
            "xyzhl": np.ascontiguousarray(xhl.reshape(A, BLOC * 6)),
        })
    return in_maps


_PROG_CACHE = {}


def _get_program(key=(None, 8)):
    if key not in _PROG_CACHE:
        _PROG_CACHE[key] = build_program(loop_iters=key[0], body_unroll=key[1])
    return _PROG_CACHE[key]


def kernel(**inputs):
    args = {k: np.asarray(v) for k, v in inputs.items()}
    in_maps = shard_inputs(args["bond_representations"], args["bond_pairs"],
                           args["bond_neighbors"], args["xyz"])
    nc = _get_program()
    res = run_bass_kernel_spmd(nc, in_maps, list(range(NCORES)))
    out = np.concatenate(
        [np.asarray(res.results[c]["out"]).astype(np.float32)
         for c in range(NCORES)], axis=0)
    return out[None]
